# revision 4
# baseline (speedup 1.0000x reference)
"""MatrixKAN layer on 8 Trainium2 NeuronCores (Bass/Tile).

Math: for a uniform extended grid, the reference's scatter of local B-spline
values into the global basis is identical to evaluating the cardinal cubic
B-spline at integer shifts:

    full[b,i,j] = N3(w[b,i] - j),   w = (x - grid[i,0]) / h_i
    N3(s) = sum_m c_m * relu(s - m)^3,  c = [1,-4,6,-4,1]/6

so with R[b,i,d] = relu(w[b,i]-d)^3 (d = 0..14) and
Ctilde[i,o,d] = sum_j coef_eff[i,o,j] * c_{d-j} (a tiny host-side weight
convolution), the spline term is one dense fp32 matmul with contraction
(i,d) = 512*15 = 7680:

    y[b,o] = sum_{i,d} R[b,i,d] Ctilde[i,o,d] + sum_i silu(x[b,i]) S[i,o]

Sharding: data-parallel over batch (8 shards of 512 rows); Ctilde/S
replicated. Device computes R via ACT (square w-d) + DVE (relu, multiply)
in transposed (i on partitions, b on free) layout, feeds the PE directly.
"""

import numpy as np

B, IN, OUT = 4096, 512, 512
NB, D = 11, 15          # nbasis, truncated-power shifts
NCORES = 8
BSH = B // NCORES       # 512 batch rows per core
IBLK = IN // 128        # 4 partition blocks of input dims
MBLK = BSH // 128       # 4 batch blocks per core

_cache = {}


def _build_program():
    import concourse.bass as bass
    import concourse.bacc as bacc
    import concourse.mybir as mybir
    import concourse.tile as tile

    f32 = mybir.dt.float32
    Act = mybir.ActivationFunctionType
    Alu = mybir.AluOpType

    nc = bacc.Bacc()
    xt_d = nc.declare_dram_parameter("xt", (IN, BSH), f32, isOutput=False)
    ct_d = nc.declare_dram_parameter("ct", (D * IN, OUT), f32, isOutput=False)
    sb_d = nc.declare_dram_parameter("sb", (IN, OUT), f32, isOutput=False)
    hs_d = nc.declare_dram_parameter("hs", (IBLK, 128, 2), f32, isOutput=False)
    nd_d = nc.declare_dram_parameter("negd", (128, D), f32, isOutput=False)
    y_d = nc.declare_dram_parameter("y", (BSH, OUT), f32, isOutput=True)

    with tile.TileContext(nc) as tc:
        with (
            tc.tile_pool(name="const", bufs=1) as constp,
            tc.tile_pool(name="xp", bufs=4) as xp,
            tc.tile_pool(name="wp", bufs=4) as wp,
            tc.tile_pool(name="silup", bufs=IBLK) as silup,
            tc.tile_pool(name="sqp", bufs=4) as sqp,
            tc.tile_pool(name="rlp", bufs=4) as rlp,
            tc.tile_pool(name="r3p", bufs=24) as r3p,
            tc.tile_pool(name="ctp", bufs=8) as ctp,
            tc.tile_pool(name="sbp", bufs=IBLK) as sbp,
            tc.tile_pool(name="yp", bufs=MBLK) as yp,
            tc.tile_pool(name="psum", bufs=MBLK, space=bass.MemorySpace.PSUM) as psp,
        ):
            hs_t = constp.tile([128, 2 * IBLK], f32)
            for blk in range(IBLK):
                nc.sync.dma_start(hs_t[:, 2 * blk:2 * blk + 2], hs_d[blk])
            nd_t = constp.tile([128, D], f32)
            nc.sync.dma_start(nd_t[:], nd_d[:])

            # base-part weights can load immediately
            sb_ts = []
            for blk in range(IBLK):
                t = sbp.tile([128, OUT], f32, tag="sb", name=f"sb{blk}")
                nc.sync.dma_start(t[:], sb_d[128 * blk:128 * (blk + 1), :])
                sb_ts.append(t)

            psums = [psp.tile([128, OUT], f32, tag="ps", name=f"ps{m}") for m in range(MBLK)]
            silus = []

            for blk in range(IBLK):
                x_t = xp.tile([128, BSH], f32, tag="x")
                nc.sync.dma_start(x_t[:], xt_d[128 * blk:128 * (blk + 1), :])

                ct_ts = []
                for d in range(D):
                    t = ctp.tile([128, OUT], f32, tag="ct", name=f"ct{blk}_{d}")
                    r0 = d * IN + 128 * blk
                    nc.sync.dma_start(t[:], ct_d[r0:r0 + 128, :])
                    ct_ts.append(t)

                inv_h = hs_t[:, 2 * blk:2 * blk + 1]
                nw0 = hs_t[:, 2 * blk + 1:2 * blk + 2]
                w_t = wp.tile([128, BSH], f32, tag="w")
                nc.scalar.activation(w_t[:], x_t[:], Act.Identity,
                                     bias=nw0, scale=inv_h)
                silu_t = silup.tile([128, BSH], f32, tag="silu")
                nc.scalar.activation(silu_t[:], x_t[:], Act.Silu)
                silus.append(silu_t)

                for d in range(D):
                    sq = sqp.tile([128, BSH], f32, tag="sq")
                    nc.scalar.activation(sq[:], w_t[:], Act.Square,
                                         bias=nd_t[:, d:d + 1])
                    rl = rlp.tile([128, BSH], f32, tag="rl")
                    nc.vector.tensor_scalar(rl[:], w_t[:], float(-d), 0.0,
                                            Alu.add, Alu.max)
                    r3 = r3p.tile([128, BSH], f32, tag="r3")
                    nc.vector.tensor_mul(r3[:], rl[:], sq[:])
                    for m in range(MBLK):
                        nc.tensor.matmul(
                            psums[m][:],
                            r3[:, 128 * m:128 * (m + 1)],
                            ct_ts[d][:],
                            start=(blk == 0 and d == 0),
                            stop=False,
                        )

            # base (silu) contribution: 4 more K-chunks per psum bank
            for blk in range(IBLK):
                for m in range(MBLK):
                    nc.tensor.matmul(
                        psums[m][:],
                        silus[blk][:, 128 * m:128 * (m + 1)],
                        sb_ts[blk][:],
                        start=False,
                        stop=(blk == IBLK - 1),
                    )

            for m in range(MBLK):
                y_t = yp.tile([128, OUT], f32, tag="y")
                nc.scalar.copy(y_t[:], psums[m][:])
                nc.sync.dma_start(y_d[128 * m:128 * (m + 1), :], y_t[:])

    nc.finalize()
    return nc


def _prep(inputs):
    x = np.ascontiguousarray(inputs["x"], np.float32)
    grid = np.asarray(inputs["grid"], np.float32)
    coef = np.asarray(inputs["coef"], np.float32)
    scale_base = np.asarray(inputs["scale_base"], np.float32)
    scale_sp = np.asarray(inputs["scale_sp"], np.float32)
    mask = np.asarray(inputs["mask"], np.float32)

    num = coef.shape[-1] - 3
    h = (grid[:, num + 3] - grid[:, 3]) / num          # (IN,)
    g0 = grid[:, 0]
    inv_h = (1.0 / h).astype(np.float32)
    nw0 = (-g0 / h).astype(np.float32)
    hs = np.stack([inv_h, nw0], axis=1).reshape(IBLK, 128, 2).astype(np.float32)

    c5 = (np.array([1., -4., 6., -4., 1.], np.float64) / 6.0).astype(np.float32)
    C = coef * (scale_sp * mask)[:, :, None]
    ct = np.zeros((D, IN, OUT), np.float32)
    for d in range(D):
        for j in range(max(0, d - 4), min(NB, d + 1)):
            ct[d] += C[:, :, j] * c5[d - j]
    ct = np.ascontiguousarray(ct.reshape(D * IN, OUT))
    sb = np.ascontiguousarray(scale_base * mask, dtype=np.float32)
    negd = np.broadcast_to(-np.arange(D, dtype=np.float32), (128, D)).copy()
    return x, ct, sb, hs, negd


def kernel(**inputs):
    from concourse.bass_utils import run_bass_kernel_spmd
    import os

    x, ct, sb, hs, negd = _prep(inputs)

    if "nc" not in _cache:
        _cache["nc"] = _build_program()
    nc = _cache["nc"]

    in_maps = []
    for c in range(NCORES):
        xt = np.ascontiguousarray(x[c * BSH:(c + 1) * BSH, :].T)
        in_maps.append({"xt": xt, "ct": ct, "sb": sb, "hs": hs,
                        "negd": negd})

    res = run_bass_kernel_spmd(nc, in_maps, list(range(NCORES)))
    _cache["last_results"] = res
    y = np.concatenate([res.results[c]["y"] for c in range(NCORES)], axis=0)
    return y.astype(np.float32)
